# revision 4
# baseline (speedup 1.0000x reference)
"""Trainium2 Bass kernel for DeformCapsNet (conv backbone + 2 capsule layers with
dynamic routing + SE gating). Data-parallel over batch: 1 sample per NeuronCore.

Self-contained: hardcodes all shapes; host-side numpy only rearranges weights /
im2cols the 3-channel input; all FLOPs run on device.

Algorithm notes (validated in numpy against the jax reference):
  - routing logits are constant along the atom axis -> kept as [p, o*c].
  - 3x3 convs = 9 shifted matmuls over a zero-padded 98x98 flat grid (9604
    positions, padded to 9728 = 76 tiles of 128); border positions compute
    garbage votes that are masked to zero before any reduction over p.
  - votes conv emitted in [p, o*c*a] layout (lhsT = shifted f tile, rhs = W);
    d-outer/h-inner order so each stationary f-tile serves 2 matmuls. Capsule
    conv bias is added via a vector add on PSUM only when nonzero (it is zero
    in the reference task, so the op is skipped entirely).
  - iteration i preact[oc,a] = sum_p r[p,oc] v[p,oc*16+a] computed as a full
    [64,1024] matmul (r as lhsT) accumulated in PSUM over all 76 tiles, then
    the 64 diagonal blocks are extracted via mask+strided-reduce. Iteration 1
    uses r = 1/8 uniformly (softmax of zero logits).
  - routing iterations process tiles in groups of 8: sweep 1 computes
    agree[p,oc] per tile on the vector engine (fp16 mult + fp16 grouped
    reduce, both 2x-packed) into per-layer fp16 logits; the softmax tail
    (exp on scalar, sum/recip/normalize on vector) runs batched per group;
    sweep 2 issues the preact matmuls re-reading votes from SBUF.
  - votes round-trip through HBM (fp16) for tiles not held in the SBUF cache.
  - phase interleaving: layer-1 votes conv (PE-heavy) is emitted interleaved
    with layer-0 routing iteration 2 (DVE-heavy), and layer-0 iteration 3
    with layer-1 iteration 2, so engines overlap.
"""

import numpy as np
from contextlib import ExitStack

import concourse.bass as bass
import concourse.bacc as bacc
import concourse.mybir as mybir
import concourse.tile as tile
from concourse.bass_utils import run_bass_kernel_spmd
from concourse.masks import make_identity

N_CORES = 8
PH = PW = 98
NP = PH * PW            # 9604 padded grid positions
NT = 76                 # p-tiles of 128
NPT = NT * 128          # 9728
G = 128                 # guard columns on each side of f buffers
NCACHE = 12             # votes tiles kept SBUF-resident per layer
GRP = 8                 # routing tiles per batched-softmax group
OFFS = [(dy, dx) for dy in (-1, 0, 1) for dx in (-1, 0, 1)]
DELTAS = [dy * PW + dx for dy, dx in OFFS]
F32 = mybir.dt.float32
F16 = mybir.dt.float16
AF = mybir.ActivationFunctionType
ALU = mybir.AluOpType
X_AX = mybir.AxisListType.X


def _groups():
    return [list(range(s, min(s + GRP, NT))) for s in range(0, NT, GRP)]


def _diag_mask():
    m = np.zeros((64, 1024), dtype=np.float32)
    for oc in range(64):
        m[oc, oc * 16:(oc + 1) * 16] = 1.0
    return m


def _border_mask():
    """1.0 at interior padded-flat positions, 0.0 at borders/tail. [128, NT]."""
    m = np.zeros((PH, PW), dtype=np.float32)
    m[1:-1, 1:-1] = 1.0
    flat = np.zeros(NPT, dtype=np.float32)
    flat[:NP] = m.reshape(-1)
    return flat.reshape(NT, 128).T.copy()  # [p_local, t]


def _zero_f_borders(nc, buf, nparts):
    """memset border cols of an [nparts, G+NPT+G] padded f buffer (interior part)."""
    v = buf[:nparts, :]
    nc.vector.memset(v[:, G:G + PW], 0.0)                       # top row
    nc.vector.memset(v[:, G + NP - PW:G + NPT], 0.0)            # bottom row + tail
    lc = v[:, G:G + NP].rearrange("p (r c) -> p r c", c=PW)
    nc.vector.memset(lc[:, :, 0:1], 0.0)                        # left col
    nc.vector.memset(lc[:, :, PW - 1:PW], 0.0)                  # right col


def build_program(has_capsbias):
    nc = bacc.Bacc(
        "TRN2", target_bir_lowering=False, debug=False, num_devices=N_CORES
    )

    def inp(name, shape, dt=F16):
        return nc.dram_tensor(name, shape, dt, kind="ExternalInput").ap()

    io = {
        "xdup": inp("xdup", [27, NPT]),
        "w1": inp("w1", [27, 64]),
        "b1": inp("b1", [64, 1], F32),
        "w2": inp("w2", [64, 9 * 128]),
        "b2": inp("b2", [128, 1], F32),
        "wcaps": inp("wcaps", [2, 128, 9, 1024]),
        "bcap": inp("bcap", [1, 2 * 1024], F32),
        "bias2": inp("bias2", [64, 2, 16], F32),
        "vmask": inp("vmask", [128, NT], F32),
        "dmask": inp("dmask", [64, 1024], F32),
        "se1": inp("se1", [33, 4], F32),
        "se2": inp("se2", [5, 32], F32),
        "out": nc.dram_tensor("out", [64, 32], F32, kind="ExternalOutput").ap(),
        "votes": nc.dram_tensor("votes_scratch", [2, NT, 128, 1024], F16).ap(),
        "actrep": nc.dram_tensor("actrep_scratch", [2, 2, 64, 16], F16).ap(),
    }

    with tile.TileContext(nc) as tc, ExitStack() as ctx:
        _body(ctx, tc, io, has_capsbias)
    nc.compile()
    return nc


class _State:
    pass


def _finish_iter(nc, st, l, preact_ps, act_out, act16_out):
    """preact psum [64,1024] -> diag-extract + bias -> squash -> act [64,16]."""
    pool = st.small
    b2l = st.bias2_sb[:, l * 16:(l + 1) * 16]
    masked = pool.tile([64, 1024], F32, tag="maskd")
    nc.vector.tensor_mul(masked, preact_ps, st.dmask_sb)
    pre = pool.tile([64, 16], F32, tag="pre")
    nc.vector.reduce_sum(pre, masked.rearrange("p (g a) -> p a g", a=16),
                         axis=X_AX)
    nc.vector.tensor_add(pre, pre, b2l)
    sq = pool.tile([64, 16], F32, tag="sq")
    ssum = pool.tile([64, 1], F32, tag="ssum")
    nc.scalar.activation(sq, pre, AF.Square, accum_out=ssum)
    nrm = pool.tile([64, 1], F32, tag="nrm")
    nc.scalar.activation(nrm, ssum, AF.Sqrt)
    den = pool.tile([64, 1], F32, tag="den")
    nc.vector.tensor_scalar_add(den, ssum, 1.0)
    rec = pool.tile([64, 1], F32, tag="rec")
    nc.vector.reciprocal(rec, den)
    scl = pool.tile([64, 1], F32, tag="scl")
    nc.vector.tensor_mul(scl, nrm, rec)
    nc.vector.tensor_scalar(act_out, pre, scl, None, op0=ALU.mult)
    if act16_out is not None:
        nc.vector.tensor_scalar(act16_out, pre, scl, None, op0=ALU.mult)


def _emit_conv_tile(nc, st, l, t, preact_ps):
    """Pass-1 work for p-tile t of layer l: votes conv + masked store + 1/8 preact."""
    vps = st.ps_votes.tile([128, 1024], F32, tag="vps")
    base = G + t * 128
    wl = st.wl[l]
    for i, d in enumerate(DELTAS):
        f_sl = st.f_buf[:, base + d:base + d + 128]
        for h in range(2):
            o = h * 512
            nc.tensor.matmul(vps[:, o:o + 512], f_sl,
                             wl[:, i * 1024 + o:i * 1024 + o + 512],
                             start=(i == 0), stop=(i == 8))
    if st.biasrep is not None:
        nc.vector.tensor_add(vps, vps, st.biasrep[l])
    if t < NCACHE:
        v_sb = st.vcache[l][:, t * 1024:(t + 1) * 1024]
        nc.scalar.activation(v_sb, vps, AF.Copy, scale=st.vmask_sb[:, t:t + 1])
    else:
        v_sb = st.vsb_pool.tile([128, 1024], F16, tag="vsb")
        nc.scalar.activation(v_sb, vps, AF.Copy, scale=st.vmask_sb[:, t:t + 1])
        nc.sync.dma_start(st.votes_d[l, t], v_sb)
    for h in range(2):
        o = h * 512
        nc.tensor.matmul(preact_ps[:, o:o + 512], st.eighth, v_sb[:, o:o + 512],
                         start=(t == 0), stop=(t == NT - 1))


def _route_group(nc, st, l, it, g, preact_ps, act_rep):
    """Routing iteration `it` of layer l over tile group g (batched softmax)."""
    gn = len(g)
    vs = []
    # sweep 1: agree per tile -> fp16 logits
    for t in g:
        if t < NCACHE:
            v_sb = st.vcache[l][:, t * 1024:(t + 1) * 1024]
        else:
            v_sb = st.vsb_pool.tile([128, 1024], F16, tag="vsb")
            nc.sync.dma_start(v_sb, st.votes_d[l, t])
        vs.append(v_sb)
        # av + pair-halving on gpsimd for even tiles, vector for odd (balance)
        eng = nc.gpsimd if t % 2 == 0 else nc.vector
        av = st.av_pool.tile([128, 1024], F16, tag="av")
        eng.tensor_mul(av, v_sb, act_rep)
        av4 = av.rearrange("p (g two a) -> p g two a", two=2, a=8)
        half = st.av_pool.tile([128, 512], F16, tag="avh")
        h3 = half.rearrange("p (g a) -> p g a", a=8)
        eng.tensor_add(h3, av4[:, :, 0, :], av4[:, :, 1, :])
        lsl = st.logits[l][:, t * 64:(t + 1) * 64]
        with nc.allow_low_precision(reason="fp16 agree reduce (16 atoms)"):
            if it == 2:
                nc.vector.reduce_sum(lsl, h3, axis=X_AX)
            else:
                agr = st.rt_pool.tile([128, 64], F16, tag="agr")
                nc.vector.reduce_sum(agr, h3, axis=X_AX)
                nc.vector.tensor_add(lsl, lsl, agr)
    # batched softmax tail over the group's logits
    g0 = g[0]
    lsl_g = st.logits[l][:, g0 * 64:(g0 + gn) * 64]
    e_gf = st.rt_pool.tile([128, GRP * 64], F16, tag="eg", name="e_gf")
    e_g = e_gf[:, :gn * 64]
    nc.scalar.activation(e_g, lsl_g, AF.Exp)
    s_gf = st.rt_pool.tile([128, GRP * 8], F16, tag="sg", name="s_gf")
    s_g = s_gf[:, :gn * 8]
    with nc.allow_low_precision(reason="fp16 softmax denominator (8 caps)"):
        nc.vector.reduce_sum(s_g, e_g.rearrange("p (x c) -> p x c", c=8),
                             axis=X_AX)
        rc_gf = st.rt_pool.tile([128, GRP * 8], F16, tag="rcg", name="rc_gf")
        rc_g = rc_gf[:, :gn * 8]
        nc.vector.reciprocal(rc_g, s_g)
    r_gf = st.rt_pool.tile([128, GRP * 64], F16, tag="rg", name="r_gf")
    r_g = r_gf[:, :gn * 64]
    nc.vector.tensor_mul(
        r_g.rearrange("p (x c) -> p x c", c=8),
        e_g.rearrange("p (x c) -> p x c", c=8),
        rc_g.unsqueeze(2).broadcast_to((128, gn * 8, 8)))
    # sweep 2: preact matmuls
    for k, t in enumerate(g):
        r16 = r_g[:, k * 64:(k + 1) * 64]
        for h in range(2):
            o = h * 512
            nc.tensor.matmul(preact_ps[:, o:o + 512], r16, vs[k][:, o:o + 512],
                             start=(t == 0), stop=(t == NT - 1))


def _route_prologue(nc, st, l, it, act16):
    nc.sync.dma_start(st.actrep_d[l, it - 2], act16)
    act_rep = st.small.tile([128, 1024], F16, tag="actrep")
    nc.sync.dma_start(
        act_rep,
        st.actrep_d[l, it - 2].rearrange("p a -> (p a)").unsqueeze(0)
        .broadcast_to((128, 1024)))
    return act_rep


def _body(ctx, tc, io, has_capsbias):
    nc = tc.nc
    st = _State()
    persist = ctx.enter_context(tc.tile_pool(name="persist", bufs=1))
    st.small = ctx.enter_context(tc.tile_pool(name="small", bufs=2))
    st.vsb_pool = ctx.enter_context(tc.tile_pool(name="vsb", bufs=20))
    st.av_pool = ctx.enter_context(tc.tile_pool(name="av", bufs=4))
    st.rt_pool = ctx.enter_context(tc.tile_pool(name="rt", bufs=3))
    wl_pool = ctx.enter_context(tc.tile_pool(name="wl", bufs=1))
    st.votes_d = io["votes"]
    st.actrep_d = io["actrep"]

    # ---- persistent tensors / constants ----
    st.f_buf = persist.tile([128, G + NPT + G], F16)
    st.logits = [persist.tile([128, NT * 64], F16, name=f"logits{l}")
                 for l in range(2)]
    w1_sb = persist.tile([27, 64], F16)
    b1_sb = persist.tile([64, 1], F32)
    w2_sb = persist.tile([64, 9 * 128], F16)
    b2_sb = persist.tile([128, 1], F32)
    st.bias2_sb = persist.tile([64, 2 * 16], F32)
    st.vmask_sb = persist.tile([128, NT], F32)
    st.dmask_sb = persist.tile([64, 1024], F32)
    se1_sb = persist.tile([33, 4], F32)
    se2_sb = persist.tile([5, 32], F32)
    st.eighth = persist.tile([128, 64], F16)
    ident = persist.tile([128, 128], F32)
    comb = persist.tile([64, 32], F32)
    if has_capsbias:
        st.biasrep = [persist.tile([128, 1024], F32, name=f"biasrep{l}")
                      for l in range(2)]
        for l in range(2):
            nc.sync.dma_start(
                st.biasrep[l],
                io["bcap"][:, l * 1024:(l + 1) * 1024]
                .broadcast_to((128, 1024)))
    else:
        st.biasrep = None
    # layer weights both resident (fp16): 2 x 18KB/partition
    st.wl = [wl_pool.tile([128, 9 * 1024], F16, tag=f"wl{l}", name=f"wl{l}")
             for l in range(2)]

    for name, sb in [("w1", w1_sb), ("b1", b1_sb), ("w2", w2_sb), ("b2", b2_sb),
                     ("vmask", st.vmask_sb), ("dmask", st.dmask_sb),
                     ("se1", se1_sb), ("se2", se2_sb)]:
        nc.sync.dma_start(sb, io[name])
    nc.sync.dma_start(st.bias2_sb, io["bias2"].rearrange("p l a -> p (l a)"))
    for l in range(2):
        nc.sync.dma_start(st.wl[l], io["wcaps"][l].rearrange("k i n -> k (i n)"))
    nc.vector.memset(st.eighth, 0.125)
    make_identity(nc, ident)
    nc.vector.memset(st.f_buf[:, 0:G], 0.0)
    nc.vector.memset(st.f_buf[:, G + NPT:], 0.0)

    # ---- backbone ----
    with tc.tile_pool(name="backbone", bufs=1) as bb, \
         tc.tile_pool(name="psb", bufs=2, space="PSUM") as psb:
        xdup_sb = bb.tile([27, NPT], F16)
        f1_buf = bb.tile([64, G + NPT + G], F16)
        nc.sync.dma_start(xdup_sb, io["xdup"])
        nc.vector.memset(f1_buf[:, 0:G], 0.0)
        nc.vector.memset(f1_buf[:, G + NPT:], 0.0)

        for t in range(NPT // 512):
            ps = psb.tile([64, 512], F32, tag="c1")
            nc.tensor.matmul(ps, w1_sb, xdup_sb[:, t * 512:(t + 1) * 512],
                             start=True, stop=True)
            nc.scalar.activation(f1_buf[:, G + t * 512:G + (t + 1) * 512], ps,
                                 AF.Relu, bias=b1_sb)
        _zero_f_borders(nc, f1_buf, 64)

        for t in range(NPT // 512):
            ps = psb.tile([128, 512], F32, tag="c2")
            base = G + t * 512
            for i, d in enumerate(DELTAS):
                nc.tensor.matmul(
                    ps, w2_sb[:, i * 128:(i + 1) * 128],
                    f1_buf[:, base + d:base + d + 512],
                    start=(i == 0), stop=(i == 8))
            nc.scalar.activation(st.f_buf[:, base:base + 512], ps, AF.Relu,
                                 bias=b2_sb)
        _zero_f_borders(nc, st.f_buf, 128)

    # ---- capsule layers (phase-interleaved) ----
    caps_ctx = ExitStack()
    vcache_pool = caps_ctx.enter_context(tc.tile_pool(name="vcache", bufs=1))
    st.vcache = [vcache_pool.tile([128, NCACHE * 1024], F16, name=f"vcache{l}")
                 for l in range(2)]
    st.ps_votes = caps_ctx.enter_context(
        tc.tile_pool(name="psv", bufs=2, space="PSUM"))
    ps_pre = caps_ctx.enter_context(
        tc.tile_pool(name="psp", bufs=2, space="PSUM"))

    def new_act(l):
        a = st.small.tile([64, 16], F32, tag="act", name=f"act{l}")
        a16 = st.small.tile([64, 16], F16, tag="act16", name=f"act16{l}")
        return a, a16

    # phase A: layer 0 pass 1
    pre_a = ps_pre.tile([64, 1024], F32, tag="pre", bufs=1)
    for t in range(NT):
        _emit_conv_tile(nc, st, 0, t, pre_a)
    act0 = new_act(0)
    _finish_iter(nc, st, 0, pre_a, act0[0], act0[1])

    # phase B: layer 1 pass 1 interleaved with layer 0 iteration 2
    pre_b1 = ps_pre.tile([64, 1024], F32, tag="pre", bufs=1)
    pre_b0 = ps_pre.tile([64, 1024], F32, tag="pre2", bufs=1)
    rep = _route_prologue(nc, st, 0, 2, act0[1])
    for g in _groups():
        for t in g:
            _emit_conv_tile(nc, st, 1, t, pre_b1)
        _route_group(nc, st, 0, 2, g, pre_b0, rep)
    act1 = new_act(1)
    _finish_iter(nc, st, 1, pre_b1, act1[0], act1[1])
    act0 = new_act(0)
    _finish_iter(nc, st, 0, pre_b0, act0[0], act0[1])

    # phase C: layer 0 iteration 3 interleaved with layer 1 iteration 2
    pre_c0 = ps_pre.tile([64, 1024], F32, tag="pre", bufs=1)
    pre_c1 = ps_pre.tile([64, 1024], F32, tag="pre2", bufs=1)
    rep0 = _route_prologue(nc, st, 0, 3, act0[1])
    rep1 = _route_prologue(nc, st, 1, 2, act1[1])
    for g in _groups():
        _route_group(nc, st, 0, 3, g, pre_c0, rep0)
        _route_group(nc, st, 1, 2, g, pre_c1, rep1)
    _finish_iter(nc, st, 0, pre_c0, comb[:, 0:16], None)
    act1 = new_act(1)
    _finish_iter(nc, st, 1, pre_c1, act1[0], act1[1])

    # phase D: layer 1 iteration 3
    pre_d = ps_pre.tile([64, 1024], F32, tag="pre", bufs=1)
    rep = _route_prologue(nc, st, 1, 3, act1[1])
    for g in _groups():
        _route_group(nc, st, 1, 3, g, pre_d, rep)
    _finish_iter(nc, st, 1, pre_d, comb[:, 16:32], None)

    caps_ctx.close()

    # ---- SE block ----
    with tc.tile_pool(name="se", bufs=1) as se, \
         tc.tile_pool(name="pse", bufs=1, space="PSUM") as pse:
        ctp = pse.tile([32, 64], F32)
        nc.tensor.transpose(ctp, comb, ident[:64, :64])
        ct = se.tile([33, 64], F32)
        nc.vector.memset(ct, 1.0)
        nc.vector.tensor_copy(ct[:32, :], ctp)
        e1p = pse.tile([64, 4], F32)
        nc.tensor.matmul(e1p, ct, se1_sb, start=True, stop=True)
        e1 = se.tile([64, 4], F32)
        nc.scalar.activation(e1, e1p, AF.Relu)
        e1tp = pse.tile([4, 64], F32)
        nc.tensor.transpose(e1tp, e1, ident[:64, :64])
        e1t = se.tile([5, 64], F32)
        nc.vector.memset(e1t, 1.0)
        nc.vector.tensor_copy(e1t[:4, :], e1tp)
        e2p = pse.tile([64, 32], F32)
        nc.tensor.matmul(e2p, e1t, se2_sb, start=True, stop=True)
        e2 = se.tile([64, 32], F32)
        nc.scalar.activation(e2, e2p, AF.Sigmoid)
        out_sb = se.tile([64, 32], F32)
        nc.vector.tensor_mul(out_sb, e2, comb)
        nc.sync.dma_start(io["out"], out_sb)


def host_inputs(x, conv1_w, conv1_b, conv2_w, conv2_b, inst_w, inst_b, inst_bias,
                cls_w, cls_b, cls_bias, se_w1, se_b1, se_w2, se_b2):
    """Host-side rearrangement of inputs into the kernel's DRAM layouts."""
    f4, f2 = np.float32, np.float16
    B = x.shape[0]
    xp = np.zeros((B, 3, PH, PW), f4)
    xp[:, :, 1:-1, 1:-1] = x
    xg = np.zeros((B, 3, 99 + NPT + 99), f4)
    xg[:, :, 99:99 + NP] = xp.reshape(B, 3, NP)
    xdup = np.empty((B, 27, NPT), f2)
    for i, d in enumerate(DELTAS):
        xdup[:, 3 * i:3 * i + 3, :] = xg[:, :, 99 + d:99 + d + NPT]

    w1 = np.ascontiguousarray(conv1_w.transpose(2, 3, 1, 0).reshape(27, 64)).astype(f2)
    w2 = np.ascontiguousarray(
        conv2_w.transpose(1, 2, 3, 0).reshape(64, 9 * 128)).astype(f2)
    wcaps = np.stack([
        np.ascontiguousarray(w.transpose(1, 2, 3, 0).reshape(128, 9, 1024))
        for w in (inst_w, cls_w)]).astype(f2)
    bcap = np.concatenate([inst_b, cls_b]).reshape(1, 2048).astype(f4)
    bias2 = np.stack([inst_bias.reshape(64, 16), cls_bias.reshape(64, 16)],
                     axis=1).astype(f4)  # [64, 2, 16]
    se1 = np.concatenate([se_w1.T, se_b1[None, :]], 0).astype(f4)
    se2 = np.concatenate([se_w2.T, se_b2[None, :]], 0).astype(f4)

    shared = {
        "w1": w1, "b1": conv1_b.reshape(64, 1).astype(f4),
        "w2": w2, "b2": conv2_b.reshape(128, 1).astype(f4),
        "wcaps": wcaps, "bcap": bcap, "bias2": bias2,
        "vmask": _border_mask(),
        "dmask": _diag_mask(),
        "se1": se1, "se2": se2,
    }
    return [dict(shared, xdup=np.ascontiguousarray(xdup[b])) for b in range(B)]


_NC_CACHE = {}


def _program(has_capsbias):
    if has_capsbias not in _NC_CACHE:
        _NC_CACHE[has_capsbias] = build_program(has_capsbias)
    return _NC_CACHE[has_capsbias]


def kernel(**inputs):
    inputs = {k: np.asarray(v, dtype=np.float32) for k, v in inputs.items()}
    has_capsbias = bool(np.any(inputs["inst_b"]) or np.any(inputs["cls_b"]))
    in_maps = host_inputs(**inputs)
    nc = _program(has_capsbias)
    res = run_bass_kernel_spmd(nc, in_maps, core_ids=list(range(N_CORES)))
    return np.stack([res.results[b]["out"].reshape(8, 8, 32)
                     for b in range(N_CORES)])
